# revision 1
# baseline (speedup 1.0000x reference)
"""Trainium2 Bass kernel for nn_End2EndRVFixedOutput (nms_detection).

Reference semantics: out[100,7] starts at zeros; for n = 0..7 in order,
with off_n = (0 if n==0 else num_dets[n-1]) and k_n = num_dets[n],
rows [off_n, off_n+k_n) are overwritten with
[n, boxes[n,j,0:4], classes[n,j], scores[n,j]] for j = row-off_n.

Since num_dets < 12, only the first 12 rows of each batch matter, and only
out rows 0..21 can ever be written.  The device kernel therefore DMAs just
the [:, :12] slices from DRAM and evaluates the ragged overwrite as a
masked matmul:

  rm8[n,r]   = (r >= off_n) & (r < off_n+k_n)            # batch n covers row r
  effT[n,r]  = rm8[n,r] * (no m>n covers r)              # n is the LAST writer
  G[12n+j,r] = (r-j == off_n) * (j < k_n) * effT[n,r]    # one-hot gather matrix
  out[r,c]   = sum_p G[p,r] * x7stack[p,c]               # exact: one term/entry

All comparisons are against per-partition scalars derived from num_dets on
device; the few data-independent helper matrices (iotas / selection
matrices) are passed in as one constant blob.  Every core runs the full
(tiny) computation on replicated inputs; core 0's output is returned.
"""

import sys

import numpy as np

_TRN_REPO = "/opt/trn_rl_repo"
if _TRN_REPO not in sys.path:
    sys.path.insert(0, _TRN_REPO)

import concourse.bacc as bacc
import concourse.bass as bass
import concourse.mybir as mybir
import concourse.tile as tile
from concourse.bass_utils import run_bass_kernel_spmd

B = 8          # batches
N_FULL = 8192  # detections per batch in the full input
J = 12         # num_dets < 12, so only rows [:12] of each batch matter
R = 100        # fixed output rows
P96 = B * J    # 96 stacked (batch, j) rows

F32 = mybir.dt.float32
I32 = mybir.dt.int32

# Constant blob layout (float32):
#   CA [8, 212]  = R8[8,100] | U8[8,8] | SH8[8,8] | SEL96[8,96]
#   CB [96, 102] = CT96[96,100] | J96[96,1] | VD96[96,1]
CA_COLS = R + 8 + 8 + P96          # 212
CB_COLS = R + 1 + 1                # 102
CONST_LEN = 8 * CA_COLS + P96 * CB_COLS


def _make_consts() -> np.ndarray:
    r = np.arange(R, dtype=np.float32)
    m = np.arange(8)
    R8 = np.tile(r[None, :], (8, 1))                                   # R8[n,r] = r
    U8 = (m[:, None] > m[None, :]).astype(np.float32)                  # U8[m,n] = m>n
    SH8 = (m[:, None] == m[None, :] - 1).astype(np.float32)            # SH8[m,p] = m==p-1
    p = np.arange(P96)
    SEL96 = (m[:, None] == p[None, :] // J).astype(np.float32)         # SEL96[m,p] = m==n(p)
    CA = np.concatenate([R8, U8, SH8, SEL96], axis=1)                  # [8, 212]
    jj = (p % J).astype(np.float32)
    nn = (p // J).astype(np.float32)
    CT96 = r[None, :] - jj[:, None]                                    # CT96[p,r] = r-j
    CB = np.concatenate([CT96, jj[:, None], nn[:, None]], axis=1)      # [96, 102]
    blob = np.concatenate([CA.ravel(), CB.ravel()]).astype(np.float32)
    assert blob.shape == (CONST_LEN,)
    return np.ascontiguousarray(blob)


def _build_nc() -> bass.Bass:
    nc = bacc.Bacc(None, target_bir_lowering=False)
    nd_d = nc.dram_tensor("num_dets", [B], I32, kind="ExternalInput")
    boxes_d = nc.dram_tensor("boxes", [B, N_FULL, 4], F32, kind="ExternalInput")
    scores_d = nc.dram_tensor("scores", [B, N_FULL], F32, kind="ExternalInput")
    classes_d = nc.dram_tensor("classes", [B, N_FULL], F32, kind="ExternalInput")
    const_d = nc.dram_tensor("consts", [CONST_LEN], F32, kind="ExternalInput")
    out_d = nc.dram_tensor("out", [R, 7], F32, kind="ExternalOutput")

    with tile.TileContext(nc) as tc:
        with (
            tc.tile_pool(name="sb", bufs=1) as sb,
            tc.tile_pool(name="ps", bufs=1, space=bass.MemorySpace.PSUM) as ps,
        ):
            ca = sb.tile([8, CA_COLS], F32)
            cb = sb.tile([P96, CB_COLS], F32)
            ndi = sb.tile([B, 1], I32)
            bx = sb.tile([P96, 4], F32)
            cl = sb.tile([P96, 1], F32)
            sc = sb.tile([P96, 1], F32)
            k8 = sb.tile([8, 1], F32)
            s8 = sb.tile([8, 1], F32)
            offk8 = sb.tile([8, 2], F32)
            v8 = sb.tile([8, R], F32)
            rm8 = sb.tile([8, R], F32)
            effT = sb.tile([8, R], F32)
            jl96 = sb.tile([P96, 1], F32)
            of96 = sb.tile([P96, 1], F32)
            m96 = sb.tile([P96, R], F32)
            g96 = sb.tile([P96, R], F32)
            outs = sb.tile([R, 7], F32)

            offp = ps.tile([8, 1], F32)
            stn = ps.tile([8, R], F32)
            ofkp = ps.tile([P96, 2], F32)
            effp = ps.tile([P96, R], F32)
            outp = ps.tile([R, 7], F32)

            dma = nc.sync.dma_start
            # constant blob
            dma(out=ca[:], in_=const_d[0 : 8 * CA_COLS].rearrange("(p f) -> p f", p=8))
            dma(
                out=cb[:],
                in_=const_d[8 * CA_COLS : CONST_LEN].rearrange("(p f) -> p f", p=P96),
            )
            # input slices straight from the full DRAM tensors
            dma(out=ndi[:], in_=nd_d[:].rearrange("(p f) -> p f", f=1))
            dma(out=bx[:], in_=boxes_d[:, 0:J, :])
            dma(out=cl[:], in_=classes_d[:, 0:J])
            dma(out=sc[:], in_=scores_d[:, 0:J])

            R8 = ca[:, 0:R]
            U8 = ca[:, R : R + 8]
            SH8 = ca[:, R + 8 : R + 16]
            SEL96 = ca[:, R + 16 : R + 16 + P96]
            CT96 = cb[:, 0:R]
            J96 = cb[:, R : R + 1]
            VD96 = cb[:, R + 1 : R + 2]

            alu = mybir.AluOpType
            vec = nc.vector

            # k8[n] = float(num_dets[n])
            vec.tensor_copy(k8[:], ndi[:])
            # offp[n] = num_dets[n-1] (0 for n=0)
            nc.tensor.matmul(offp[:], SH8, k8[:], start=True, stop=True)
            # s8 = off + k;  offk8 = [off | k]
            vec.tensor_scalar(s8[:], offp[:], k8[:], None, alu.add)
            vec.tensor_copy(offk8[:, 0:1], offp[:])
            vec.tensor_copy(offk8[:, 1:2], k8[:])
            # rm8[n,r] = (r >= off_n) & (r < off_n + k_n)
            vec.tensor_scalar(v8[:], R8, s8[:], None, alu.is_lt)
            vec.scalar_tensor_tensor(
                rm8[:], R8, offk8[:, 0:1], v8[:], alu.is_ge, alu.mult
            )
            # stn[n,r] = sum_{m>n} rm8[m,r];  effT = (stn==0) * rm8
            nc.tensor.matmul(stn[:], U8, rm8[:], start=True, stop=True)
            vec.scalar_tensor_tensor(
                effT[:], stn[:], 0.0, rm8[:], alu.is_equal, alu.mult
            )
            # per-(n,j)-row broadcasts of off_n / k_n
            nc.tensor.matmul(ofkp[:], SEL96, offk8[:], start=True, stop=True)
            vec.tensor_scalar(jl96[:], ofkp[:, 1:2], J96, None, alu.is_gt)
            vec.tensor_copy(of96[:], ofkp[:, 0:1])
            # m96[12n+j, r] = (r-j == off_n) * (j < k_n)
            vec.tensor_scalar(m96[:], CT96, of96[:], jl96[:], alu.is_equal, alu.mult)
            # effp[12n+j, r] = effT[n, r];  g96 = m96 * effp
            nc.tensor.matmul(effp[:], SEL96, effT[:], start=True, stop=True)
            vec.tensor_tensor(g96[:], m96[:], effp[:], alu.mult)
            # out[r, :] = G.T @ [vd | boxes | classes | scores]
            nc.tensor.matmul(outp[:, 0:1], g96[:], VD96, start=True, stop=True)
            nc.tensor.matmul(outp[:, 1:5], g96[:], bx[:], start=True, stop=True)
            nc.tensor.matmul(outp[:, 5:6], g96[:], cl[:], start=True, stop=True)
            nc.tensor.matmul(outp[:, 6:7], g96[:], sc[:], start=True, stop=True)
            vec.tensor_copy(outs[:], outp[:])
            dma(out=out_d[:], in_=outs[:])

    nc.finalize()
    return nc


_CACHE: dict = {}


def _get_built():
    if "nc" not in _CACHE:
        _CACHE["nc"] = _build_nc()
        _CACHE["consts"] = _make_consts()
    return _CACHE["nc"], _CACHE["consts"]


def run(inputs: dict, trace: bool = False, **spmd_kwargs):
    """Run on all 8 cores with replicated inputs; returns (out, BassKernelResults)."""
    nc, consts = _get_built()
    in_map = {
        "num_dets": np.ascontiguousarray(inputs["num_dets"], dtype=np.int32),
        "boxes": np.ascontiguousarray(inputs["boxes"], dtype=np.float32),
        "scores": np.ascontiguousarray(inputs["scores"], dtype=np.float32),
        "classes": np.ascontiguousarray(inputs["classes"], dtype=np.float32),
        "consts": consts,
    }
    res = run_bass_kernel_spmd(
        nc,
        [dict(in_map) for _ in range(8)],
        core_ids=list(range(8)),
        trace=trace,
        **spmd_kwargs,
    )
    return res.results[0]["out"], res


def kernel(num_dets, boxes, scores, classes):
    out, _ = run(
        {"num_dets": num_dets, "boxes": boxes, "scores": scores, "classes": classes}
    )
    return out
